# revision 33
# baseline (speedup 1.0000x reference)
"""Trainium2 Bass kernel for nn_AttentionForONNX (Transformer-XL style
relative-position attention).

Strategy (data-parallel over batch, 2 batches per core on 8 cores):
  - Host pre-transposes x -> xT [B,E,T], pos_emb -> peT [B,E,S2+1 padded] and
    weights so all on-chip matmuls contract over the partition dimension.
  - Projections computed transposed: qT/kT = W @ x.T (+bias per-partition),
    v natural = x @ Wv.T + bv (bf16), pT = Wp @ pos_emb.T.
  - Scores per (t_tile, head): ac = quT-slice x kT; bd band U [128,639]
    over the needed relative-position window; the rel_shift is a *diagonal
    SBUF->SBUF DMA* (per-partition source offset via a flat access pattern
    with row step 639 on a 640-pitch tile) -- zero compute cost.
  - softmax without max-subtraction (logits are O(4), fp32-safe):
    ACT exp (bf16 out) with fused row-sum accumulation; normalization is a
    per-partition reciprocal multiply on the *output* (natural orientation).
  - attn @ v: PE-transpose exp tiles (bf16 pass-through, exact), then bf16
    matmuls with transposed-attn as stationary -> out[t, e] natural.
  - Projection matmuls in float32r (tf32-like, 1 cycle/row at even N>=256).
"""
import sys
import os

for _p in ("/opt/trn_rl_repo", "/root/.axon_site/_ro/trn_rl_repo"):
    if os.path.isdir(_p) and _p not in sys.path:
        sys.path.insert(0, _p)

import numpy as np

B, T, E, H = 16, 512, 512, 8
HD = E // H
S2 = 2 * T - 1
N_CORES = 8
BPC = B // N_CORES          # batches per core
SCALE = 1.0 / float(np.sqrt(HD))

_CACHE = {}


def _split_multiwaits(nc, mybir):
    """walrus supports only one sync-wait per instruction: split extras
    into single-wait NOPs preceding the instruction."""
    n = 0
    for bb in nc.main_func.blocks:
        new_insts = []
        for ins in bb.instructions:
            si = ins.sync_info
            if si and si.on_wait and len(si.on_wait) > 1:
                waits = list(si.on_wait)
                for w in waits[:-1]:
                    nop = mybir.InstNoOp(name=f"{ins.name}-w{n}", ins=[], outs=[])
                    nop.engine = ins.engine
                    nop.sync_info = mybir.SyncInfo(on_wait=[w], on_update=[])
                    nc.register_instruction(nop, overwrite=True)
                    new_insts.append(nop)
                    n += 1
                ins.sync_info = mybir.SyncInfo(on_wait=[waits[-1]],
                                               on_update=list(si.on_update))
            new_insts.append(ins)
        bb.instructions[:] = new_insts
    return n


def _build_nc(bpc=BPC):
    import concourse.bass as bass
    import concourse.mybir as mybir
    import concourse.tile as tile
    from concourse.ap import AP
    from concourse.masks import make_identity

    F32 = mybir.dt.float32
    F32R = mybir.dt.float32r
    BF16 = mybir.dt.bfloat16
    AT = mybir.AluOpType
    AF = mybir.ActivationFunctionType

    nc = bass.Bass("TRN2", target_bir_lowering=False)

    xT = nc.dram_tensor("xT", [bpc, E, T], F32R, kind="ExternalInput")
    peT = nc.dram_tensor("peT", [bpc, E, S2 + 1], F32R, kind="ExternalInput")
    wqT = nc.dram_tensor("wqT", [E, E], F32R, kind="ExternalInput")
    wkT = nc.dram_tensor("wkT", [E, E], F32R, kind="ExternalInput")
    wvT = nc.dram_tensor("wvT", [E, E], F32R, kind="ExternalInput")
    wpT = nc.dram_tensor("wpT", [E, E], F32R, kind="ExternalInput")
    # bias_u/bias_v/bk packed [128, 4]: column eo = bias[eo*128:(eo+1)*128]
    bu = nc.dram_tensor("bu", [128, 4], F32, kind="ExternalInput")
    bvv = nc.dram_tensor("bvv", [128, 4], F32, kind="ExternalInput")
    bkk = nc.dram_tensor("bkk", [128, 4], F32, kind="ExternalInput")
    bvec = nc.dram_tensor("bvec", [E], F32, kind="ExternalInput")   # bv for v
    out_d = nc.dram_tensor("out", [bpc, T, E], F32, kind="ExternalOutput")

    NEO = E // 128   # 4 tiles along e_out / e_in / s / t
    J0 = [384 - 128 * tt for tt in range(4)]

    with tile.TileContext(nc) as tc:
        with (
            tc.tile_pool(name="const", bufs=1) as const,
            tc.tile_pool(name="batch", bufs=1) as batch,
            tc.tile_pool(name="blate", bufs=2) as blate,
            tc.tile_pool(name="work", bufs=4) as work,
            tc.tile_pool(name="et", bufs=2) as etp,
            tc.tile_pool(name="pps", bufs=2, space="PSUM") as pps,     # 4 banks
            tc.tile_pool(name="acps", bufs=2, space="PSUM") as acps,   # 2 banks
            tc.tile_pool(name="tps", bufs=1, space="PSUM") as tps,     # 1 bank
            tc.tile_pool(name="avps", bufs=1, space="PSUM") as avps,   # 1 bank
        ):
            # ---- tiny constants first: the projection epilogues (bias adds)
            # gate on these; don't queue 6KB behind 13MB of weights
            BU = const.tile([128, 4], F32, tag="bu")
            nc.sync.dma_start(out=BU, in_=bu[:])
            BV = const.tile([128, 4], F32, tag="bvv")
            nc.sync.dma_start(out=BV, in_=bvv[:])
            BK = const.tile([128, 4], F32, tag="bkk")
            nc.sync.dma_start(out=BK, in_=bkk[:])
            BVB = const.tile([128, E], F32, tag="bvb")
            nc.sync.dma_start(out=BVB, in_=AP(bvec, 0, [[0, 128], [1, E]]))
            IDENT = const.tile([128, 128], BF16, tag="ident")
            make_identity(nc, IDENT[:])
            # ---- batch-0 inputs first (the first projection MMs need
            # xT[0]+Wq[0]; don't queue them behind 12MB of weights)
            XT0 = [batch.tile([128, T], F32R, tag=f"xT{ei}", name=f"xT0{ei}")
                   for ei in range(NEO)]
            for ei in range(NEO):
                nc.sync.dma_start(out=XT0[ei],
                                  in_=xT[0, ei * 128:(ei + 1) * 128, :])
            # ---- constants
            W = {}
            for name, drt in (("q", wqT), ("k", wkT)):
                W[name] = [const.tile([128, E], F32R, tag=f"w{name}{ei}",
                                      name=f"w{name}{ei}")
                           for ei in range(NEO)]
                for ei in range(NEO):
                    nc.sync.dma_start(out=W[name][ei],
                                      in_=drt[ei * 128:(ei + 1) * 128, :])
            for name, drt in (("v", wvT),):
                W[name] = [const.tile([128, E], F32R, tag=f"w{name}{ei}",
                                      name=f"w{name}{ei}")
                           for ei in range(NEO)]
                for ei in range(NEO):
                    nc.sync.dma_start(out=W[name][ei],
                                      in_=drt[ei * 128:(ei + 1) * 128, :])
            PET0 = [batch.tile([128, S2 + 1], F32R, tag=f"peT{ei}",
                                name=f"peT0{ei}")
                    for ei in range(NEO)]
            for ei in range(NEO):
                nc.sync.dma_start(out=PET0[ei],
                                  in_=peT[0, ei * 128:(ei + 1) * 128, :])
            for name, drt in (("p", wpT),):
                W[name] = [const.tile([128, E], F32R, tag=f"w{name}{ei}",
                                      name=f"w{name}{ei}")
                           for ei in range(NEO)]
                for ei in range(NEO):
                    nc.sync.dma_start(out=W[name][ei],
                                      in_=drt[ei * 128:(ei + 1) * 128, :])

            def emit_loads(b):
                XT = [batch.tile([128, T], F32R, tag=f"xT{ei}",
                                 name=f"xTs{b}_{ei}")
                      for ei in range(NEO)]
                PET = [batch.tile([128, S2 + 1], F32R, tag=f"peT{ei}",
                                  name=f"peTs{b}_{ei}")
                       for ei in range(NEO)]
                for ei in range(NEO):
                    nc.sync.dma_start(out=XT[ei],
                                      in_=xT[b, ei * 128:(ei + 1) * 128, :])
                    nc.sync.dma_start(out=PET[ei],
                                      in_=peT[b, ei * 128:(ei + 1) * 128, :])
                return XT, PET

            def emit_proj(b, XT, PET):
                QU, QV, KT, VSB, PT = [], [], [], [], []
                for eo in range(NEO):
                    pq = pps.tile([128, S2 + 1], F32, tag="pp", name="pq")
                    for ei in range(NEO):
                        nc.tensor.matmul(
                            pq[:, 0:T], W["q"][ei][:, eo * 128:(eo + 1) * 128],
                            XT[ei][:], start=(ei == 0), stop=(ei == NEO - 1))
                    qu = blate.tile([128, T], F32R, tag=f"qu{eo}", name=f"qu{eo}")
                    nc.scalar.activation(qu[:], pq[:, 0:T], AF.Identity,
                                         bias=BU[:, eo:eo + 1])
                    qv = blate.tile([128, T], F32R, tag=f"qv{eo}", name=f"qv{eo}")
                    nc.scalar.activation(qv[:], pq[:, 0:T], AF.Identity,
                                         bias=BV[:, eo:eo + 1])
                    QU.append(qu)
                    QV.append(qv)
                    pk = pps.tile([128, S2 + 1], F32, tag="pp", name="pk")
                    for ei in range(NEO):
                        nc.tensor.matmul(
                            pk[:, 0:T], W["k"][ei][:, eo * 128:(eo + 1) * 128],
                            XT[ei][:], start=(ei == 0), stop=(ei == NEO - 1))
                    kt = blate.tile([128, T], F32R, tag=f"kt{eo}", name=f"kt{eo}")
                    nc.scalar.activation(kt[:], pk[:, 0:T], AF.Identity,
                                         bias=BK[:, eo:eo + 1])
                    KT.append(kt)
                for st in range(NEO):
                    pv = pps.tile([128, S2 + 1], F32, tag="pp", name="pv")
                    for ei in range(NEO):
                        nc.tensor.matmul(
                            pv[:, 0:E], XT[ei][:, st * 128:(st + 1) * 128],
                            W["v"][ei][:], start=(ei == 0), stop=(ei == NEO - 1))
                    vsb = blate.tile([128, E], BF16, tag=f"v{st}", name=f"v{st}")
                    nc.vector.tensor_tensor(vsb[:], pv[:, 0:E], BVB[:], AT.add)
                    VSB.append(vsb)
                for eo in range(NEO):
                    pp = pps.tile([128, S2 + 1], F32, tag="pp", name="ppp")
                    for ei in range(NEO):
                        nc.tensor.matmul(
                            pp[:, 0:512], W["p"][ei][:, eo * 128:(eo + 1) * 128],
                            PET[ei][:, 0:512], start=(ei == 0),
                            stop=(ei == NEO - 1))
                    for ei in range(NEO):
                        nc.tensor.matmul(
                            pp[:, 512:S2 + 1],
                            W["p"][ei][:, eo * 128:(eo + 1) * 128],
                            PET[ei][:, 512:S2 + 1], start=(ei == 0),
                            stop=(ei == NEO - 1))
                    pt = blate.tile([128, S2 + 1], F32R, tag=f"pt{eo}",
                                    name=f"pt{eo}")
                    nc.vector.tensor_copy(pt[:], pp[:])
                    PT.append(pt)
                return QU, QV, KT, VSB, PT

            def emit_attn_tt(b, tt, QU, QV, KT, VSB, PT):
                j0 = J0[tt]
                DENS = []
                ETS = []
                for h in range(H):
                    hp, r0 = h // 2, 64 * (h % 2)
                    lqv = QV[hp][r0:r0 + 64, 128 * tt:128 * (tt + 1)]
                    u = pps.tile([128, S2 + 1], F32, tag="pp", name="u")
                    nc.tensor.matmul(u[:, 0:512], lqv,
                                     PT[hp][r0:r0 + 64, j0:j0 + 512],
                                     start=True, stop=True,
                                     tile_position=(r0, 0))
                    nc.tensor.matmul(u[:, 512:640], lqv,
                                     PT[hp][r0:r0 + 64, j0 + 512:j0 + 640],
                                     start=True, stop=True,
                                     tile_position=(r0, 0))
                    ub = work.tile([128, 640], BF16, tag="ub", name="ub")
                    if h % 5 == 2:
                        nc.scalar.activation(ub[:, 0:639], u[:, 0:639],
                                             AF.Copy)
                    else:
                        nc.vector.tensor_copy(ub[:, 0:639], u[:, 0:639])
                    v_ = work.tile([128, T], BF16, tag="vv", name="vv")
                    nc.sync.dma_start(
                        out=v_,
                        in_=AP(ub.tensor, 127, [[639, 128], [1, 512]]))
                    # psum-preload the shifted bd via identity matmul, then
                    # the ac matmul ACCUMULATES on top: no DVE add needed
                    pac = acps.tile([128, T], F32, tag="pac", name="pac")
                    nc.tensor.matmul(pac[:], IDENT[:], v_[:],
                                     start=True, stop=False)
                    lq = QU[hp][r0:r0 + 64, 128 * tt:128 * (tt + 1)]
                    nc.tensor.matmul(pac[:], lq, KT[hp][r0:r0 + 64, :],
                                     start=False, stop=True,
                                     tile_position=(r0, 0))
                    den = work.tile([128, 1], F32, tag=f"den{h}",
                                    name=f"den{h}")
                    DENS.append(den)
                    e_ = work.tile([128, T], BF16, tag="ee", name="ee")
                    nc.scalar.activation(e_[:], pac[:], AF.Exp,
                                         bias=0.0, scale=SCALE,
                                         accum_out=den[:])
                    tp = tps.tile([128, T], BF16, tag="tp", name="tp")
                    for k in range(4):
                        nc.tensor.transpose(
                            tp[:, 128 * k:128 * (k + 1)],
                            e_[:, 128 * k:128 * (k + 1)], IDENT[:])
                    ett = etp.tile([128, T], BF16, tag=f"ett{h}",
                                   name=f"ett{h}")
                    nc.vector.tensor_copy(ett[:], tp[:])
                    ETS.append(ett)
                RECS = []
                for h in range(H):
                    rec = work.tile([128, 1], F32, tag=f"rec{h}",
                                    name=f"rec{h}")
                    nc.vector.reciprocal(rec[:], DENS[h][:])
                    RECS.append(rec)
                av = avps.tile([128, E], F32, tag="av", name="av")
                for h in range(H):
                    for k in range(4):
                        nc.tensor.matmul(
                            av[:, 64 * h:64 * (h + 1)],
                            ETS[h][:, 128 * k:128 * (k + 1)],
                            VSB[k][:, 64 * h:64 * (h + 1)],
                            start=(k == 0), stop=(k == 3))
                o_sb = work.tile([128, E], F32, tag="osb", name="osb")
                for h in range(H):
                    if h % 2 == 0:
                        nc.vector.tensor_scalar_mul(
                            o_sb[:, 64 * h:64 * (h + 1)],
                            av[:, 64 * h:64 * (h + 1)], RECS[h][:])
                    else:
                        nc.scalar.activation(
                            o_sb[:, 64 * h:64 * (h + 1)],
                            av[:, 64 * h:64 * (h + 1)], AF.Copy,
                            scale=RECS[h][:])
                nc.sync.dma_start(
                    out=out_d[b, 128 * tt:128 * (tt + 1), :], in_=o_sb)

            # software-pipelined schedule: batch b+1's projections are
            # emitted after batch b's first attention t_tile so they fill
            # PE/ACT gaps during b's attention phase.
            ctx0 = emit_proj(0, XT0, PET0)
            pend = {}
            for b in range(bpc):
                ctx = ctx0 if b == 0 else pend.pop(b)
                for tt in range(4):
                    emit_attn_tt(b, tt, *ctx)
                    if tt == 0 and b + 1 < bpc:
                        XTn, PETn = emit_loads(b + 1)
                        pend[b + 1] = emit_proj(b + 1, XTn, PETn)

    _split_multiwaits(nc, mybir)
    return nc


def _prep_inputs(x, pos_emb, Wq, bq, Wk, bk, Wv, bv, Wp,
                 pos_bias_u, pos_bias_v):
    xT = np.ascontiguousarray(np.asarray(x, np.float32).transpose(0, 2, 1))
    peT = np.zeros((B, E, S2 + 1), np.float32)
    peT[:, :, 0:S2] = np.asarray(pos_emb, np.float32).transpose(0, 2, 1)
    wqT = np.ascontiguousarray(np.asarray(Wq, np.float32).T)
    wkT = np.ascontiguousarray(np.asarray(Wk, np.float32).T)
    wvT = np.ascontiguousarray(np.asarray(Wv, np.float32).T)
    wpT = np.ascontiguousarray(np.asarray(Wp, np.float32).T)
    bias_u = (np.asarray(bq, np.float32)
              + np.asarray(pos_bias_u, np.float32).reshape(E))
    bias_v = (np.asarray(bq, np.float32)
              + np.asarray(pos_bias_v, np.float32).reshape(E))
    bu_p = np.ascontiguousarray(bias_u.reshape(4, 128).T)
    bv_p = np.ascontiguousarray(bias_v.reshape(4, 128).T)
    bk_p = np.ascontiguousarray(np.asarray(bk, np.float32).reshape(4, 128).T)
    common = {
        "wqT": wqT, "wkT": wkT, "wvT": wvT, "wpT": wpT,
        "bu": bu_p, "bvv": bv_p, "bkk": bk_p,
        "bvec": np.asarray(bv, np.float32),
    }
    in_maps = []
    for c in range(N_CORES):
        m = dict(common)
        m["xT"] = xT[c * BPC:(c + 1) * BPC]
        m["peT"] = peT[c * BPC:(c + 1) * BPC]
        in_maps.append(m)
    return in_maps


def kernel(x, pos_emb, Wq, bq, Wk, bk, Wv, bv, Wp,
           pos_bias_u, pos_bias_v, legacy=0, **_):
    from concourse.bass_utils import run_bass_kernel_spmd

    if "nc" not in _CACHE:
        _CACHE["nc"] = _build_nc()
    nc = _CACHE["nc"]
    in_maps = _prep_inputs(x, pos_emb, Wq, bq, Wk, bk, Wv, bv, Wp,
                           pos_bias_u, pos_bias_v)
    res = run_bass_kernel_spmd(nc, in_maps, list(range(N_CORES))).results
    return np.concatenate([r["out"] for r in res], axis=0)   # [B, T, E]


# revision 34
# speedup vs baseline: 1.0085x; 1.0085x over previous
"""Trainium2 Bass kernel for nn_AttentionForONNX (Transformer-XL style
relative-position attention).

Strategy (data-parallel over batch, 2 batches per core on 8 cores):
  - Host pre-transposes x -> xT [B,E,T], pos_emb -> peT [B,E,S2+1 padded] and
    weights so all on-chip matmuls contract over the partition dimension.
  - Projections computed transposed: qT/kT = W @ x.T (+bias per-partition),
    v natural = x @ Wv.T + bv (bf16), pT = Wp @ pos_emb.T.
  - Scores per (t_tile, head): ac = quT-slice x kT; bd band U [128,639]
    over the needed relative-position window; the rel_shift is a *diagonal
    SBUF->SBUF DMA* (per-partition source offset via a flat access pattern
    with row step 639 on a 640-pitch tile) -- zero compute cost.
  - softmax without max-subtraction (logits are O(4), fp32-safe):
    ACT exp (bf16 out) with fused row-sum accumulation; normalization is a
    per-partition reciprocal multiply on the *output* (natural orientation).
  - attn @ v: PE-transpose exp tiles (bf16 pass-through, exact), then bf16
    matmuls with transposed-attn as stationary -> out[t, e] natural.
  - Projection matmuls in float32r (tf32-like, 1 cycle/row at even N>=256).
"""
import sys
import os

for _p in ("/opt/trn_rl_repo", "/root/.axon_site/_ro/trn_rl_repo"):
    if os.path.isdir(_p) and _p not in sys.path:
        sys.path.insert(0, _p)

import numpy as np

B, T, E, H = 16, 512, 512, 8
HD = E // H
S2 = 2 * T - 1
N_CORES = 8
BPC = B // N_CORES          # batches per core
SCALE = 1.0 / float(np.sqrt(HD))

_CACHE = {}


def _split_multiwaits(nc, mybir):
    """walrus supports only one sync-wait per instruction: split extras
    into single-wait NOPs preceding the instruction."""
    n = 0
    for bb in nc.main_func.blocks:
        new_insts = []
        for ins in bb.instructions:
            si = ins.sync_info
            if si and si.on_wait and len(si.on_wait) > 1:
                waits = list(si.on_wait)
                for w in waits[:-1]:
                    nop = mybir.InstNoOp(name=f"{ins.name}-w{n}", ins=[], outs=[])
                    nop.engine = ins.engine
                    nop.sync_info = mybir.SyncInfo(on_wait=[w], on_update=[])
                    nc.register_instruction(nop, overwrite=True)
                    new_insts.append(nop)
                    n += 1
                ins.sync_info = mybir.SyncInfo(on_wait=[waits[-1]],
                                               on_update=list(si.on_update))
            new_insts.append(ins)
        bb.instructions[:] = new_insts
    return n


def _build_nc(bpc=BPC):
    import concourse.bass as bass
    import concourse.mybir as mybir
    import concourse.tile as tile
    from concourse.ap import AP
    from concourse.masks import make_identity

    F32 = mybir.dt.float32
    F32R = mybir.dt.float32r
    BF16 = mybir.dt.bfloat16
    AT = mybir.AluOpType
    AF = mybir.ActivationFunctionType

    nc = bass.Bass("TRN2", target_bir_lowering=False)

    xT = nc.dram_tensor("xT", [bpc, E, T], F32R, kind="ExternalInput")
    peT = nc.dram_tensor("peT", [bpc, E, S2 + 1], F32R, kind="ExternalInput")
    wqT = nc.dram_tensor("wqT", [E, E], F32R, kind="ExternalInput")
    wkT = nc.dram_tensor("wkT", [E, E], F32R, kind="ExternalInput")
    wvT = nc.dram_tensor("wvT", [E, E], F32R, kind="ExternalInput")
    wpT = nc.dram_tensor("wpT", [E, E], F32R, kind="ExternalInput")
    # bias_u/bias_v/bk packed [128, 4]: column eo = bias[eo*128:(eo+1)*128]
    bu = nc.dram_tensor("bu", [128, 4], F32, kind="ExternalInput")
    bvv = nc.dram_tensor("bvv", [128, 4], F32, kind="ExternalInput")
    bkk = nc.dram_tensor("bkk", [128, 4], F32, kind="ExternalInput")
    bvec = nc.dram_tensor("bvec", [E], F32, kind="ExternalInput")   # bv for v
    out_d = nc.dram_tensor("out", [bpc, T, E], F32, kind="ExternalOutput")

    NEO = E // 128   # 4 tiles along e_out / e_in / s / t
    J0 = [384 - 128 * tt for tt in range(4)]

    with tile.TileContext(nc) as tc:
        with (
            tc.tile_pool(name="const", bufs=1) as const,
            tc.tile_pool(name="batch", bufs=1) as batch,
            tc.tile_pool(name="blate", bufs=2) as blate,
            tc.tile_pool(name="work", bufs=4) as work,
            tc.tile_pool(name="et", bufs=2) as etp,
            tc.tile_pool(name="pps", bufs=2, space="PSUM") as pps,     # 4 banks
            tc.tile_pool(name="acps", bufs=2, space="PSUM") as acps,   # 2 banks
            tc.tile_pool(name="tps", bufs=1, space="PSUM") as tps,     # 1 bank
            tc.tile_pool(name="avps", bufs=1, space="PSUM") as avps,   # 1 bank
        ):
            # ---- tiny constants first: the projection epilogues (bias adds)
            # gate on these; don't queue 6KB behind 13MB of weights
            BU = const.tile([128, 4], F32, tag="bu")
            nc.sync.dma_start(out=BU, in_=bu[:])
            BV = const.tile([128, 4], F32, tag="bvv")
            nc.sync.dma_start(out=BV, in_=bvv[:])
            BK = const.tile([128, 4], F32, tag="bkk")
            nc.sync.dma_start(out=BK, in_=bkk[:])
            BVB = const.tile([128, E], F32, tag="bvb")
            nc.sync.dma_start(out=BVB, in_=AP(bvec, 0, [[0, 128], [1, E]]))
            IDENT = const.tile([128, 128], BF16, tag="ident")
            make_identity(nc, IDENT[:])
            # ---- batch-0 inputs first (the first projection MMs need
            # xT[0]+Wq[0]; don't queue them behind 12MB of weights)
            XT0 = [batch.tile([128, T], F32R, tag=f"xT{ei}", name=f"xT0{ei}")
                   for ei in range(NEO)]
            for ei in range(NEO):
                nc.sync.dma_start(out=XT0[ei],
                                  in_=xT[0, ei * 128:(ei + 1) * 128, :])
            # ---- constants
            W = {}
            for name, drt in (("q", wqT), ("k", wkT)):
                W[name] = [const.tile([128, E], F32R, tag=f"w{name}{ei}",
                                      name=f"w{name}{ei}")
                           for ei in range(NEO)]
                for ei in range(NEO):
                    nc.sync.dma_start(out=W[name][ei],
                                      in_=drt[ei * 128:(ei + 1) * 128, :])
            for name, drt in (("v", wvT),):
                W[name] = [const.tile([128, E], F32R, tag=f"w{name}{ei}",
                                      name=f"w{name}{ei}")
                           for ei in range(NEO)]
                for ei in range(NEO):
                    nc.sync.dma_start(out=W[name][ei],
                                      in_=drt[ei * 128:(ei + 1) * 128, :])
            PET0 = [batch.tile([128, S2 + 1], F32R, tag=f"peT{ei}",
                                name=f"peT0{ei}")
                    for ei in range(NEO)]
            for ei in range(NEO):
                nc.sync.dma_start(out=PET0[ei],
                                  in_=peT[0, ei * 128:(ei + 1) * 128, :])
            for name, drt in (("p", wpT),):
                W[name] = [const.tile([128, E], F32R, tag=f"w{name}{ei}",
                                      name=f"w{name}{ei}")
                           for ei in range(NEO)]
                for ei in range(NEO):
                    nc.sync.dma_start(out=W[name][ei],
                                      in_=drt[ei * 128:(ei + 1) * 128, :])

            def emit_loads(b):
                XT = [batch.tile([128, T], F32R, tag=f"xT{ei}",
                                 name=f"xTs{b}_{ei}")
                      for ei in range(NEO)]
                PET = [batch.tile([128, S2 + 1], F32R, tag=f"peT{ei}",
                                  name=f"peTs{b}_{ei}")
                       for ei in range(NEO)]
                for ei in range(NEO):
                    nc.sync.dma_start(out=XT[ei],
                                      in_=xT[b, ei * 128:(ei + 1) * 128, :])
                    nc.sync.dma_start(out=PET[ei],
                                      in_=peT[b, ei * 128:(ei + 1) * 128, :])
                return XT, PET

            def emit_proj(b, XT, PET):
                QU, QV, KT, VSB, PT = [], [], [], [], []
                for eo in range(NEO):
                    pq = pps.tile([128, S2 + 1], F32, tag="pp", name="pq")
                    for ei in range(NEO):
                        nc.tensor.matmul(
                            pq[:, 0:T], W["q"][ei][:, eo * 128:(eo + 1) * 128],
                            XT[ei][:], start=(ei == 0), stop=(ei == NEO - 1))
                    qu = blate.tile([128, T], F32R, tag=f"qu{eo}", name=f"qu{eo}")
                    nc.scalar.activation(qu[:], pq[:, 0:T], AF.Identity,
                                         bias=BU[:, eo:eo + 1])
                    qv = blate.tile([128, T], F32R, tag=f"qv{eo}", name=f"qv{eo}")
                    nc.scalar.activation(qv[:], pq[:, 0:T], AF.Identity,
                                         bias=BV[:, eo:eo + 1])
                    QU.append(qu)
                    QV.append(qv)
                    pk = pps.tile([128, S2 + 1], F32, tag="pp", name="pk")
                    for ei in range(NEO):
                        nc.tensor.matmul(
                            pk[:, 0:T], W["k"][ei][:, eo * 128:(eo + 1) * 128],
                            XT[ei][:], start=(ei == 0), stop=(ei == NEO - 1))
                    kt = blate.tile([128, T], F32R, tag=f"kt{eo}", name=f"kt{eo}")
                    nc.scalar.activation(kt[:], pk[:, 0:T], AF.Identity,
                                         bias=BK[:, eo:eo + 1])
                    KT.append(kt)
                for st in range(NEO):
                    pv = pps.tile([128, S2 + 1], F32, tag="pp", name="pv")
                    for ei in range(NEO):
                        nc.tensor.matmul(
                            pv[:, 0:E], XT[ei][:, st * 128:(st + 1) * 128],
                            W["v"][ei][:], start=(ei == 0), stop=(ei == NEO - 1))
                    vsb = blate.tile([128, E], BF16, tag=f"v{st}", name=f"v{st}")
                    nc.vector.tensor_tensor(vsb[:], pv[:, 0:E], BVB[:], AT.add)
                    VSB.append(vsb)
                for eo in range(NEO):
                    pp = pps.tile([128, S2 + 1], F32, tag="pp", name="ppp")
                    for ei in range(NEO):
                        nc.tensor.matmul(
                            pp[:, 0:512], W["p"][ei][:, eo * 128:(eo + 1) * 128],
                            PET[ei][:, 0:512], start=(ei == 0),
                            stop=(ei == NEO - 1))
                    for ei in range(NEO):
                        nc.tensor.matmul(
                            pp[:, 512:S2 + 1],
                            W["p"][ei][:, eo * 128:(eo + 1) * 128],
                            PET[ei][:, 512:S2 + 1], start=(ei == 0),
                            stop=(ei == NEO - 1))
                    pt = blate.tile([128, S2 + 1], F32R, tag=f"pt{eo}",
                                    name=f"pt{eo}")
                    nc.vector.tensor_copy(pt[:], pp[:])
                    PT.append(pt)
                return QU, QV, KT, VSB, PT

            def emit_attn_tt(b, tt, QU, QV, KT, VSB, PT):
                j0 = J0[tt]
                DENS = []
                ETS = []
                for h in range(H):
                    hp, r0 = h // 2, 64 * (h % 2)
                    lqv = QV[hp][r0:r0 + 64, 128 * tt:128 * (tt + 1)]
                    u = pps.tile([128, S2 + 1], F32, tag="pp", name="u")
                    nc.tensor.matmul(u[:, 0:512], lqv,
                                     PT[hp][r0:r0 + 64, j0:j0 + 512],
                                     start=True, stop=True,
                                     tile_position=(r0, 0))
                    nc.tensor.matmul(u[:, 512:640], lqv,
                                     PT[hp][r0:r0 + 64, j0 + 512:j0 + 640],
                                     start=True, stop=True,
                                     tile_position=(r0, 0))
                    ub = work.tile([128, 640], BF16, tag="ub", name="ub")
                    if h % 5 == 2:
                        nc.scalar.activation(ub[:, 0:639], u[:, 0:639],
                                             AF.Copy)
                    else:
                        nc.vector.tensor_copy(ub[:, 0:639], u[:, 0:639])
                    v_ = work.tile([128, T], BF16, tag="vv", name="vv")
                    nc.sync.dma_start(
                        out=v_,
                        in_=AP(ub.tensor, 127, [[639, 128], [1, 512]]))
                    # psum-preload the shifted bd via identity matmul, then
                    # the ac matmul ACCUMULATES on top: no DVE add needed
                    pac = acps.tile([128, T], F32, tag="pac", name="pac")
                    lq = QU[hp][r0:r0 + 64, 128 * tt:128 * (tt + 1)]
                    nc.tensor.matmul(pac[:], lq, KT[hp][r0:r0 + 64, :],
                                     start=True, stop=False,
                                     tile_position=(r0, 0))
                    nc.tensor.matmul(pac[:], IDENT[:], v_[:],
                                     start=False, stop=True)
                    den = work.tile([128, 1], F32, tag=f"den{h}",
                                    name=f"den{h}")
                    DENS.append(den)
                    e_ = work.tile([128, T], BF16, tag="ee", name="ee")
                    nc.scalar.activation(e_[:], pac[:], AF.Exp,
                                         bias=0.0, scale=SCALE,
                                         accum_out=den[:])
                    tp = tps.tile([128, T], BF16, tag="tp", name="tp")
                    for k in range(4):
                        nc.tensor.transpose(
                            tp[:, 128 * k:128 * (k + 1)],
                            e_[:, 128 * k:128 * (k + 1)], IDENT[:])
                    ett = etp.tile([128, T], BF16, tag=f"ett{h}",
                                   name=f"ett{h}")
                    nc.vector.tensor_copy(ett[:], tp[:])
                    ETS.append(ett)
                RECS = []
                for h in range(H):
                    rec = work.tile([128, 1], F32, tag=f"rec{h}",
                                    name=f"rec{h}")
                    nc.vector.reciprocal(rec[:], DENS[h][:])
                    RECS.append(rec)
                av = avps.tile([128, E], F32, tag="av", name="av")
                for h in range(H):
                    for k in range(4):
                        nc.tensor.matmul(
                            av[:, 64 * h:64 * (h + 1)],
                            ETS[h][:, 128 * k:128 * (k + 1)],
                            VSB[k][:, 64 * h:64 * (h + 1)],
                            start=(k == 0), stop=(k == 3))
                o_sb = work.tile([128, E], F32, tag="osb", name="osb")
                for h in range(H):
                    if h % 2 == 0:
                        nc.vector.tensor_scalar_mul(
                            o_sb[:, 64 * h:64 * (h + 1)],
                            av[:, 64 * h:64 * (h + 1)], RECS[h][:])
                    else:
                        nc.scalar.activation(
                            o_sb[:, 64 * h:64 * (h + 1)],
                            av[:, 64 * h:64 * (h + 1)], AF.Copy,
                            scale=RECS[h][:])
                nc.sync.dma_start(
                    out=out_d[b, 128 * tt:128 * (tt + 1), :], in_=o_sb)

            # software-pipelined schedule: batch b+1's projections are
            # emitted after batch b's first attention t_tile so they fill
            # PE/ACT gaps during b's attention phase.
            ctx0 = emit_proj(0, XT0, PET0)
            pend = {}
            for b in range(bpc):
                ctx = ctx0 if b == 0 else pend.pop(b)
                for tt in range(4):
                    emit_attn_tt(b, tt, *ctx)
                    if tt == 0 and b + 1 < bpc:
                        XTn, PETn = emit_loads(b + 1)
                        pend[b + 1] = emit_proj(b + 1, XTn, PETn)

    _split_multiwaits(nc, mybir)
    return nc


def _prep_inputs(x, pos_emb, Wq, bq, Wk, bk, Wv, bv, Wp,
                 pos_bias_u, pos_bias_v):
    xT = np.ascontiguousarray(np.asarray(x, np.float32).transpose(0, 2, 1))
    peT = np.zeros((B, E, S2 + 1), np.float32)
    peT[:, :, 0:S2] = np.asarray(pos_emb, np.float32).transpose(0, 2, 1)
    wqT = np.ascontiguousarray(np.asarray(Wq, np.float32).T)
    wkT = np.ascontiguousarray(np.asarray(Wk, np.float32).T)
    wvT = np.ascontiguousarray(np.asarray(Wv, np.float32).T)
    wpT = np.ascontiguousarray(np.asarray(Wp, np.float32).T)
    bias_u = (np.asarray(bq, np.float32)
              + np.asarray(pos_bias_u, np.float32).reshape(E))
    bias_v = (np.asarray(bq, np.float32)
              + np.asarray(pos_bias_v, np.float32).reshape(E))
    bu_p = np.ascontiguousarray(bias_u.reshape(4, 128).T)
    bv_p = np.ascontiguousarray(bias_v.reshape(4, 128).T)
    bk_p = np.ascontiguousarray(np.asarray(bk, np.float32).reshape(4, 128).T)
    common = {
        "wqT": wqT, "wkT": wkT, "wvT": wvT, "wpT": wpT,
        "bu": bu_p, "bvv": bv_p, "bkk": bk_p,
        "bvec": np.asarray(bv, np.float32),
    }
    in_maps = []
    for c in range(N_CORES):
        m = dict(common)
        m["xT"] = xT[c * BPC:(c + 1) * BPC]
        m["peT"] = peT[c * BPC:(c + 1) * BPC]
        in_maps.append(m)
    return in_maps


def kernel(x, pos_emb, Wq, bq, Wk, bk, Wv, bv, Wp,
           pos_bias_u, pos_bias_v, legacy=0, **_):
    from concourse.bass_utils import run_bass_kernel_spmd

    if "nc" not in _CACHE:
        _CACHE["nc"] = _build_nc()
    nc = _CACHE["nc"]
    in_maps = _prep_inputs(x, pos_emb, Wq, bq, Wk, bk, Wv, bv, Wp,
                           pos_bias_u, pos_bias_v)
    res = run_bass_kernel_spmd(nc, in_maps, list(range(N_CORES))).results
    return np.concatenate([r["out"] for r in res], axis=0)   # [B, T, E]
